# revision 1
# baseline (speedup 1.0000x reference)
"""NonLocalBlock (GroupNorm + single-head 4096x4096 attention + residual)
Trainium2 Bass kernel, data-parallel over batch: 1 image per NeuronCore x8.

Per image (x: [512, 4096] channels-major):
  pass0: GroupNorm stats (bn_stats per channel, group-combine via tiny matmuls)
  passA: per hw-chunk of 512: normalize -> q/k (fp32 matmul, split to fp16
         hi/lo pairs), vT (fp32r). k hi/lo resident in SBUF, vT resident,
         q hi/lo spilled to DRAM.
  attention per group of 4 q-tiles (128 rows each):
         logits = 3-pass fp16-split matmul (hi*hi + hi*lo + lo*hi), softmax
         via ACT exp (accum_out row sums, fp32r probs), PE-transpose probs
         (fp32r), attn@v fp32r, transpose attn_h, batched output projection
         fp32r at N=512, +bias +residual, contiguous stores.
  bv is folded into attn_h (softmax weights sum to 1).
"""
import sys

sys.path.insert(0, '/opt/trn_rl_repo')
import numpy as np
import concourse.bass as bass
import concourse.bacc as bacc
import concourse.mybir as mybir
import concourse.tile as tile
from concourse.bass_utils import run_bass_kernel_spmd

F32 = mybir.dt.float32
F32R = mybir.dt.float32r
F16 = mybir.dt.float16
AF = mybir.ActivationFunctionType
AX = mybir.AxisListType
OP = mybir.AluOpType

C = 512
HW = 4096
NT = 4            # channel tiles of 128
NCH = 8           # hw chunks of 512
NQT = 32          # q tiles of 128
NG = 8            # groups of 4 q-tiles
GSIZE = 16        # channels per group
EPS = 1e-5
SCALE = float(np.float32(512.0) ** 0.5)


def build():
    nc = bacc.Bacc('TRN2', target_bir_lowering=False, debug=False)

    x_in = nc.declare_dram_parameter("x", [C, HW], F32, isOutput=False)
    wqh_in = nc.declare_dram_parameter("wqTh", [C, C], F16, isOutput=False)
    wql_in = nc.declare_dram_parameter("wqTl", [C, C], F16, isOutput=False)
    wkh_in = nc.declare_dram_parameter("wkTh", [C, C], F16, isOutput=False)
    wkl_in = nc.declare_dram_parameter("wkTl", [C, C], F16, isOutput=False)
    wvT_in = nc.declare_dram_parameter("wvT", [C, C], F32, isOutput=False)
    woT_in = nc.declare_dram_parameter("woT", [C, C], F32, isOutput=False)
    bias_in = nc.declare_dram_parameter("biases", [128, 16], F32,
                                        isOutput=False)  # bq|bk|bv|bo as [128,4]
    gb_in = nc.declare_dram_parameter("gammabeta", [128, 8], F32,
                                      isOutput=False)  # gamma|beta as [128,4]
    brow_in = nc.declare_dram_parameter("bias_rows", [2, C], F32,
                                        isOutput=False)  # bq|bk natural order
    out_dram = nc.declare_dram_parameter("out", [C, HW], F32, isOutput=True)

    qhi_dram = nc.dram_tensor("qhi_scratch", [C, HW], F16)
    qlo_dram = nc.dram_tensor("qlo_scratch", [C, HW], F16)

    a16 = np.zeros((128, 8), np.float32)
    for p in range(128):
        a16[p, p // GSIZE] = 1.0 / GSIZE
    b8 = np.zeros((8, 128), np.float32)
    for p in range(128):
        b8[p // GSIZE, p] = 1.0
    a16_d = nc.inline_tensor(a16, "a16")
    b8_d = nc.inline_tensor(b8, "b8")
    ident_d = nc.inline_tensor(np.eye(128, dtype=np.float32), "ident128")
    ones_d = nc.inline_tensor(np.ones((1, 512), np.float32), "ones512")

    with tile.TileContext(nc) as tc:
        with (
            tc.tile_pool(name="res", bufs=1) as res,
            tc.tile_pool(name="pp_proj", bufs=2, space="PSUM") as pp_proj,
            tc.tile_pool(name="pp_log", bufs=3, space="PSUM") as pp_log,
            tc.tile_pool(name="pp_tr", bufs=2, space="PSUM") as pp_tr,
            tc.tile_pool(name="pp_attn", bufs=1, space="PSUM") as pp_attn,
        ):
            # ---------- residents ----------
            khi_res = [res.tile([128, HW], F16, tag=f"khi{t}", name=f"khi{t}")
                       for t in range(NT)]
            klo_res = [res.tile([128, HW], F16, tag=f"klo{t}", name=f"klo{t}")
                       for t in range(NT)]
            vT_res = [res.tile([128, C], F32R, tag=f"vT{m}", name=f"vT{m}")
                      for m in range(NQT)]
            wo_sb = [res.tile([128, C], F32R, tag=f"wo{t}", name=f"wo{t}")
                     for t in range(NT)]
            for t in range(NT):
                nc.gpsimd.dma_start(out=wo_sb[t],
                                    in_=woT_in[128 * t:128 * (t + 1), :])
            biases = res.tile([128, 16], F32, tag="biases")
            nc.sync.dma_start(out=biases, in_=bias_in[:])
            bq = biases[:, 0:4]
            bk = biases[:, 4:8]
            bv = biases[:, 8:12]
            bo = biases[:, 12:16]
            gmbt = res.tile([128, 8], F32, tag="gmbt")
            nc.sync.dma_start(out=gmbt, in_=gb_in[:])
            gam = gmbt[:, 0:4]
            bet = gmbt[:, 4:8]
            a16_sb = res.tile([128, 8], F32, tag="a16")
            nc.sync.dma_start(out=a16_sb, in_=a16_d[:])
            b8_sb = res.tile([8, 128], F32, tag="b8")
            nc.sync.dma_start(out=b8_sb, in_=b8_d[:])
            id_sb = res.tile([128, 128], F32, tag="ident")
            nc.sync.dma_start(out=id_sb, in_=ident_d[:])
            idr_sb = res.tile([128, 128], F32R, tag="identr")
            nc.gpsimd.dma_start(out=idr_sb, in_=ident_d[:])
            bq_row = res.tile([1, C], F32R, tag="bq_row")
            nc.gpsimd.dma_start(out=bq_row, in_=brow_in[0:1, :])
            bk_row = res.tile([1, C], F32R, tag="bk_row")
            nc.gpsimd.dma_start(out=bk_row, in_=brow_in[1:2, :])
            ones_row = res.tile([1, 512], F32R, tag="ones_row")
            nc.gpsimd.dma_start(out=ones_row, in_=ones_d[:])
            eps8 = res.tile([8, 1], F32, tag="eps8")
            nc.vector.memset(eps8, EPS)
            scale_sb = res.tile([128, NT], F32, tag="scale")
            shift_sb = res.tile([128, NT], F32, tag="shift")

            # PE warmup: ~5us of dummy transposes while pass0 stats run,
            # so HAM unthrottles (1.2->2.4GHz) before passA matmuls start.
            wps = pp_log.tile([128, 128], F32, tag="ps_l", name="wps")
            for _ in range(24):
                nc.tensor.transpose(wps, id_sb, id_sb)

            # ---------- pass 0: GroupNorm statistics ----------
            with tc.tile_pool(name="p0", bufs=4) as p0, \
                 tc.tile_pool(name="p0s", bufs=1) as p0s:
                st6 = p0s.tile([128, NT, NCH, 6], F32, tag="st6")
                for n in range(NCH):
                    for t in range(NT):
                        xc = p0.tile([128, 512], F32, tag="x0")
                        nc.sync.dma_start(
                            out=xc,
                            in_=x_in[128 * t:128 * (t + 1), 512 * n:512 * (n + 1)])
                        nc.vector.bn_stats(out=st6[:, t, n, :], in_=xc)
                mv = p0s.tile([128, NT, 2], F32, tag="mv")
                for t in range(NT):
                    nc.vector.bn_aggr(out=mv[:, t, :], in_=st6[:, t, :, :])
                # stats_in: cols 0-3 mean_t, cols 4-7 E[x^2]_t
                stats_in = p0s.tile([128, 8], F32, tag="stats_in")
                for t in range(NT):
                    nc.vector.tensor_copy(stats_in[:, t:t + 1], mv[:, t, 0:1])
                    nc.vector.tensor_mul(stats_in[:, 4 + t:5 + t],
                                         mv[:, t, 0:1], mv[:, t, 0:1])
                    nc.vector.tensor_add(stats_in[:, 4 + t:5 + t],
                                         stats_in[:, 4 + t:5 + t], mv[:, t, 1:2])
                ps_g = pp_proj.tile([8, 8], F32, tag="ps_proj")
                nc.tensor.matmul(ps_g, a16_sb, stats_in, start=True, stop=True)
                g_sb = p0s.tile([8, 8], F32, tag="g_sb")
                nc.vector.tensor_copy(g_sb, ps_g)
                # group var = E[x^2]_g - mean_g^2 ; rstd = exp(-0.5*ln(var+eps))
                var_g = p0s.tile([8, 4], F32, tag="var_g")
                nc.vector.tensor_mul(var_g, g_sb[:, 0:4], g_sb[:, 0:4])
                nc.vector.tensor_tensor(out=var_g, in0=g_sb[:, 4:8], in1=var_g,
                                        op=OP.subtract)
                bc_in = p0s.tile([8, 8], F32, tag="bc_in")
                nc.vector.tensor_copy(bc_in[:, 0:4], g_sb[:, 0:4])
                nc.scalar.activation(out=bc_in[:, 4:8], in_=var_g, func=AF.Ln,
                                     bias=eps8, scale=1.0)
                nc.scalar.activation(out=bc_in[:, 4:8], in_=bc_in[:, 4:8],
                                     func=AF.Exp, bias=0.0, scale=-0.5)
                ps_bc = pp_proj.tile([128, 8], F32, tag="ps_proj")
                nc.tensor.matmul(ps_bc, b8_sb, bc_in, start=True, stop=True)
                chan = p0s.tile([128, 8], F32, tag="chan")
                nc.vector.tensor_copy(chan, ps_bc)
                # scale = gamma * rstd ; shift = beta - mean*scale
                nc.vector.tensor_mul(scale_sb, gam, chan[:, 4:8])
                tmp = p0s.tile([128, NT], F32, tag="tmp")
                nc.vector.tensor_mul(tmp, chan[:, 0:4], scale_sb)
                nc.vector.tensor_tensor(out=shift_sb, in0=bet, in1=tmp,
                                        op=OP.subtract)

            # ---------- pass A: hidden -> q(hi/lo), k(hi/lo), vT ----------
            with tc.tile_pool(name="pa_w", bufs=1) as pa_w, \
                 tc.tile_pool(name="pa_x", bufs=3) as pa_x, \
                 tc.tile_pool(name="pa_h", bufs=8) as pa_h, \
                 tc.tile_pool(name="pa_hr", bufs=4) as pa_hr, \
                 tc.tile_pool(name="pa_q", bufs=2) as pa_q:
                wqh_sb = [pa_w.tile([128, C], F16, tag=f"wqh{t}", name=f"wqh{t}")
                          for t in range(NT)]
                wql_sb = [pa_w.tile([128, C], F16, tag=f"wql{t}", name=f"wql{t}")
                          for t in range(NT)]
                wkh_sb = [pa_w.tile([128, C], F16, tag=f"wkh{t}", name=f"wkh{t}")
                          for t in range(NT)]
                wkl_sb = [pa_w.tile([128, C], F16, tag=f"wkl{t}", name=f"wkl{t}")
                          for t in range(NT)]
                wv_sb = [pa_w.tile([128, C], F32R, tag=f"wv{t}", name=f"wv{t}")
                         for t in range(NT)]
                for t in range(NT):
                    sl = slice(128 * t, 128 * (t + 1))
                    nc.sync.dma_start(out=wqh_sb[t], in_=wqh_in[sl, :])
                    nc.sync.dma_start(out=wql_sb[t], in_=wql_in[sl, :])
                    nc.sync.dma_start(out=wkh_sb[t], in_=wkh_in[sl, :])
                    nc.sync.dma_start(out=wkl_sb[t], in_=wkl_in[sl, :])
                    nc.gpsimd.dma_start(out=wv_sb[t], in_=wvT_in[sl, :])
                for n in range(NCH):
                    cols = slice(512 * n, 512 * (n + 1))
                    hid = []
                    hid_r = []
                    hhi = []
                    hlo = []
                    for t in range(NT):
                        xc = pa_x.tile([128, 512], F32, tag="xA")
                        nc.sync.dma_start(
                            out=xc, in_=x_in[128 * t:128 * (t + 1), cols])
                        h = pa_h.tile([128, 512], F32, tag="hid", bufs=4)
                        nc.vector.tensor_scalar(
                            out=h, in0=xc,
                            scalar1=scale_sb[:, t:t + 1],
                            scalar2=shift_sb[:, t:t + 1],
                            op0=OP.mult, op1=OP.add)
                        hid.append(h)
                        hr = pa_hr.tile([128, 512], F32R, tag="hid_r")
                        nc.gpsimd.tensor_copy(out=hr, in_=h)
                        hid_r.append(hr)
                        hh = pa_h.tile([128, 512], F16, tag="hhi", name="hh", bufs=5)
                        nc.scalar.copy(out=hh, in_=h)
                        hhi.append(hh)
                        hl = pa_h.tile([128, 512], F16, tag="hlo", name="hl", bufs=5)
                        nc.vector.tensor_tensor(out=hl, in0=h, in1=hh,
                                                op=OP.subtract)
                        hlo.append(hl)
                    # q and k projections (fp32), then split into fp16 hi/lo
                    # vT (fp32r): out[hw_t 128, c 512] = hidden_chunk_t.T @ wvT
                    # (bv folded into attn_h later: softmax weights sum to 1)
                    for t in range(NT):
                        ps = pp_proj.tile([128, 512], F32, tag="ps_proj")
                        for kc in range(NT):
                            nc.tensor.matmul(
                                ps, hid_r[kc][:, 128 * t:128 * (t + 1)],
                                wv_sb[kc], start=(kc == 0), stop=(kc == 3))
                        nc.vector.tensor_copy(vT_res[4 * n + t], ps)

                    for (w_h, w_l, b_row, is_q) in (
                            (wqh_sb, wql_sb, bq_row, True),
                            (wkh_sb, wkl_sb, bk_row, False)):
                        for m in range(NT):
                            ms = slice(128 * m, 128 * (m + 1))
                            ps = pp_proj.tile([128, 512], F32, tag="ps_proj")
                            for kc in range(NT):
                                nc.tensor.matmul(
                                    ps, w_h[kc][:, ms], hhi[kc],
                                    start=(kc == 0), stop=False)
                            for kc in range(NT):
                                nc.tensor.matmul(
                                    ps, w_h[kc][:, ms], hlo[kc],
                                    start=False, stop=False)
                            for kc in range(NT):
                                nc.tensor.matmul(
                                    ps, w_l[kc][:, ms], hhi[kc],
                                    start=False, stop=False)
                            nc.tensor.matmul(ps, b_row[:, ms], ones_row,
                                             start=False, stop=True)
                            if is_q:
                                hi = pa_q.tile([128, 512], F16, tag="qhi_st")
                                lo = pa_q.tile([128, 512], F16, tag="qlo_st")
                            else:
                                hi = khi_res[m][:, cols]
                                lo = klo_res[m][:, cols]
                            nc.scalar.copy(out=hi, in_=ps)
                            nc.vector.tensor_tensor(out=lo, in0=ps, in1=hi,
                                                    op=OP.subtract)
                            if is_q:
                                nc.sync.dma_start(
                                    out=qhi_dram[128 * m:128 * (m + 1), cols],
                                    in_=hi)
                                nc.sync.dma_start(
                                    out=qlo_dram[128 * m:128 * (m + 1), cols],
                                    in_=lo)
            # ---------- attention ----------
            with tc.tile_pool(name="at_q", bufs=2) as at_q, \
                 tc.tile_pool(name="at_l", bufs=1) as at_l, \
                 tc.tile_pool(name="at_p", bufs=3) as at_p, \
                 tc.tile_pool(name="at_pt", bufs=3) as at_pt, \
                 tc.tile_pool(name="at_s", bufs=2) as at_s, \
                 tc.tile_pool(name="at_h4", bufs=1) as at_h4, \
                 tc.tile_pool(name="at_o", bufs=2) as at_o:
                for g in range(NG):
                    attnh4 = at_h4.tile([128, NT, 512], F32R, tag="attnh4")
                    for qq in range(4):
                        qt = 4 * g + qq
                        qcols = slice(128 * qt, 128 * (qt + 1))
                        qtile_hi = at_q.tile([128, NT, 128], F16, tag="qtile_hi")
                        nc.sync.dma_start(
                            out=qtile_hi,
                            in_=qhi_dram[:, qcols].rearrange(
                                "(t p) q -> p t q", p=128))
                        qtile_lo = at_q.tile([128, NT, 128], F16, tag="qtile_lo")
                        nc.sync.dma_start(
                            out=qtile_lo,
                            in_=qlo_dram[:, qcols].rearrange(
                                "(t p) q -> p t q", p=128))

                        logits = at_l.tile([128, HW], F32, tag="logits")
                        maxs = at_s.tile([128, NCH], F32, tag="maxs")
                        for n in range(NCH):
                            ncols = slice(512 * n, 512 * (n + 1))
                            ps_l = pp_log.tile([128, 512], F32, tag="ps_l")
                            for kc in range(NT):
                                nc.tensor.matmul(
                                    ps_l, qtile_hi[:, kc, :],
                                    khi_res[kc][:, ncols],
                                    start=(kc == 0), stop=False)
                            for kc in range(NT):
                                nc.tensor.matmul(
                                    ps_l, qtile_hi[:, kc, :],
                                    klo_res[kc][:, ncols],
                                    start=False, stop=False)
                            for kc in range(NT):
                                nc.tensor.matmul(
                                    ps_l, qtile_lo[:, kc, :],
                                    khi_res[kc][:, ncols],
                                    start=False, stop=(kc == 3))
                            nc.vector.reduce_max(out=maxs[:, n:n + 1], in_=ps_l,
                                                 axis=AX.X)
                            nc.scalar.copy(out=logits[:, ncols], in_=ps_l)
                        negmax = at_s.tile([128, 1], F32, tag="negmax")
                        nc.vector.reduce_max(out=negmax, in_=maxs, axis=AX.X,
                                             negate=True)
                        negmax_s = at_s.tile([128, 1], F32, tag="negmax_s")
                        nc.vector.tensor_scalar_mul(out=negmax_s, in0=negmax,
                                                    scalar1=SCALE)
                        sums = at_s.tile([128, NCH], F32, tag="sums")
                        ps_at = pp_attn.tile([128, C], F32, tag="ps_at")
                        for n in range(NCH):
                            probs = at_p.tile([128, 512], F32R, tag="probs")
                            nc.scalar.activation(
                                out=probs, in_=logits[:, 512 * n:512 * (n + 1)],
                                func=AF.Exp, bias=negmax_s, scale=SCALE,
                                accum_out=sums[:, n:n + 1])
                            ps_t = pp_tr.tile([128, 512], F32R, tag="ps_t")
                            for j in range(4):
                                nc.tensor.transpose(
                                    ps_t[:, 128 * j:128 * (j + 1)],
                                    probs[:, 128 * j:128 * (j + 1)], idr_sb)
                            pT = at_pt.tile([128, 512], F32R, tag="pT")
                            nc.vector.tensor_copy(pT, ps_t)
                            for j in range(4):
                                nc.tensor.matmul(
                                    ps_at, pT[:, 128 * j:128 * (j + 1)],
                                    vT_res[4 * n + j],
                                    start=(n == 0 and j == 0),
                                    stop=(n == 7 and j == 3))
                        rowsum = at_s.tile([128, 1], F32, tag="rowsum")
                        nc.vector.reduce_sum(out=rowsum, in_=sums, axis=AX.X)
                        rinv = at_s.tile([128, 1], F32, tag="rinv")
                        nc.vector.reciprocal(out=rinv, in_=rowsum)
                        attn = at_s.tile([128, C], F32, tag="attn")
                        nc.vector.tensor_scalar_mul(out=attn, in0=ps_at,
                                                    scalar1=rinv)
                        # transpose attn [q, c] -> attn_h [c, q]; add bv
                        ps_t2 = pp_tr.tile([128, 512], F32, tag="ps_t")
                        for i in range(NT):
                            nc.tensor.transpose(
                                ps_t2[:, 128 * i:128 * (i + 1)],
                                attn[:, 128 * i:128 * (i + 1)], id_sb)
                        for i in range(NT):
                            nc.vector.tensor_scalar_add(
                                out=attnh4[:, i, 128 * qq:128 * (qq + 1)],
                                in0=ps_t2[:, 128 * i:128 * (i + 1)],
                                scalar1=bv[:, i:i + 1])
                    # batched output projection (fp32r, N=512) + bias + residual
                    gcols = slice(512 * g, 512 * (g + 1))
                    for m in range(NT):
                        ps_o = pp_proj.tile([128, 512], F32, tag="ps_proj")
                        for kc in range(NT):
                            nc.tensor.matmul(
                                ps_o, wo_sb[kc][:, 128 * m:128 * (m + 1)],
                                attnh4[:, kc, :], start=(kc == 0), stop=(kc == 3))
                        o_sb = at_o.tile([128, 512], F32, tag="o_sb")
                        nc.vector.tensor_scalar_add(
                            out=o_sb, in0=ps_o, scalar1=bo[:, m:m + 1])
                        xres = at_o.tile([128, 512], F32, tag="xres")
                        nc.sync.dma_start(
                            out=xres, in_=x_in[128 * m:128 * (m + 1), gcols])
                        nc.vector.tensor_add(out=o_sb, in0=o_sb, in1=xres)
                        nc.sync.dma_start(
                            out=out_dram[128 * m:128 * (m + 1), gcols], in_=o_sb)

    nc.compile()
    return nc


_NC_CACHE = None


def _prep_inputs(inputs):
    x = np.asarray(inputs["x"], np.float32)

    def tile4(v):
        return np.asarray(v, np.float32).reshape(4, 128).T

    biases = np.concatenate(
        [tile4(inputs[k]) for k in ("bq", "bk", "bv", "bo")], axis=1)
    gb = np.concatenate(
        [tile4(inputs["gn_gamma"]), tile4(inputs["gn_beta"])], axis=1)
    def split16(w):
        hi = w.astype(np.float16)
        lo = (w - hi.astype(np.float32)).astype(np.float16)
        return hi, lo

    wqT = np.ascontiguousarray(np.asarray(inputs["wq"], np.float32).T)
    wkT = np.ascontiguousarray(np.asarray(inputs["wk"], np.float32).T)
    wqh, wql = split16(wqT)
    wkh, wkl = split16(wkT)
    shared = {
        "wqTh": wqh, "wqTl": wql,
        "wkTh": wkh, "wkTl": wkl,
        "wvT": np.ascontiguousarray(np.asarray(inputs["wv"], np.float32).T),
        "woT": np.ascontiguousarray(np.asarray(inputs["wo"], np.float32).T),
        "biases": np.ascontiguousarray(biases),
        "gammabeta": np.ascontiguousarray(gb),
        "bias_rows": np.ascontiguousarray(np.stack(
            [np.asarray(inputs["bq"], np.float32),
             np.asarray(inputs["bk"], np.float32)])),
    }
    return [dict(shared, x=np.ascontiguousarray(x[i].reshape(C, HW)))
            for i in range(x.shape[0])]


def kernel(**inputs):
    global _NC_CACHE
    if _NC_CACHE is None:
        _NC_CACHE = build()
    nc = _NC_CACHE
    x = np.asarray(inputs["x"], np.float32)
    b, c, h, w = x.shape
    in_maps = _prep_inputs(inputs)
    res = run_bass_kernel_spmd(nc, in_maps, list(range(b)))
    out = np.stack([res.results[i]["out"].reshape(c, h, w) for i in range(b)])
    return out.astype(np.float32)


if __name__ == "__main__":
    import time
    t0 = time.time()
    build()
    print(f"build ok in {time.time()-t0:.1f}s")



# revision 5
# speedup vs baseline: 1.6856x; 1.6856x over previous
"""NonLocalBlock (GroupNorm + single-head 4096x4096 attention + residual)
Trainium2 Bass kernel, data-parallel over batch: 1 image per NeuronCore x8.

All-fp16 single-pass pipeline (rel-err gate 2e-2 leaves large headroom;
measured ~9e-3 in numpy emulation):
  pass0: GroupNorm stats (bn_stats per channel, group-combine via tiny
         matmuls)  + PE warmup transposes to unthrottle HAM.
  passA: per hw-chunk of 512: h16 = (scale*x+shift) cast fp16 in one
         vector op; q/k = single-pass fp16 matmul, bias+cast on vector,
         resident in SBUF (no DRAM spill); vT fp16 resident.
  attention per q-tile of 128 rows, software-pipelined (qt's softmax/attn@v
  interleaved with qt+1's logits so the PE never idles):
    logits = 1-pass fp16 matmul into PSUM; per-chunk max (vector);
    exp directly from PSUM (chunk-local max, fp16 probs, accum row sums);
    per-chunk correction exp(m_n - M) folded with nothing else (1 vector
    mul per chunk); 1/rowsum folded into the attn-output scale instead.
    PE-transpose probs (fp16), attn@v fp16, transpose attn_h fp16,
    batched output projection fp16 at N=512, +bias +residual.
  bv folded into attn_h (softmax weights sum to 1).
"""
import sys

sys.path.insert(0, '/opt/trn_rl_repo')
import numpy as np
import concourse.bass as bass
import concourse.bacc as bacc
import concourse.mybir as mybir
import concourse.tile as tile
from concourse.bass_utils import run_bass_kernel_spmd

F32 = mybir.dt.float32
F16 = mybir.dt.float16
AF = mybir.ActivationFunctionType
AX = mybir.AxisListType
OP = mybir.AluOpType

C = 512
HW = 4096
NT = 4            # channel tiles of 128
NCH = 8           # hw chunks of 512
NQT = 32          # q tiles of 128
GSIZE = 16        # channels per group
EPS = 1e-5
SCALE = float(np.float32(512.0) ** 0.5)


def build():
    nc = bacc.Bacc('TRN2', target_bir_lowering=False, debug=False)

    x_in = nc.declare_dram_parameter("x", [C, HW], F32, isOutput=False)
    wq_in = nc.declare_dram_parameter("wqT", [C, C], F16, isOutput=False)
    wk_in = nc.declare_dram_parameter("wkT", [C, C], F16, isOutput=False)
    wv_in = nc.declare_dram_parameter("wvT", [C, C], F16, isOutput=False)
    wo_in = nc.declare_dram_parameter("woT", [C, C], F16, isOutput=False)
    bias_in = nc.declare_dram_parameter("biases", [128, 16], F32,
                                        isOutput=False)  # bq|bk|bv|bo as [128,4]
    gb_in = nc.declare_dram_parameter("gammabeta", [128, 8], F32,
                                      isOutput=False)  # gamma|beta as [128,4]
    out_dram = nc.declare_dram_parameter("out", [C, HW], F32, isOutput=True)

    a16 = np.zeros((128, 8), np.float32)
    for p in range(128):
        a16[p, p // GSIZE] = 1.0 / GSIZE
    b8 = np.zeros((8, 128), np.float32)
    for p in range(128):
        b8[p // GSIZE, p] = 1.0
    a16_d = nc.inline_tensor(a16, "a16")
    b8_d = nc.inline_tensor(b8, "b8")
    ident_d = nc.inline_tensor(np.eye(128, dtype=np.float32), "ident128")
    ident16_d = nc.inline_tensor(np.eye(128, dtype=np.float16), "ident128h")

    with tile.TileContext(nc) as tc:
        with (
            tc.tile_pool(name="res", bufs=1) as res,
            tc.tile_pool(name="pp_proj", bufs=2, space="PSUM") as pp_proj,
            tc.tile_pool(name="pp_log", bufs=3, space="PSUM") as pp_log,
            tc.tile_pool(name="pp_tr", bufs=2, space="PSUM") as pp_tr,
            tc.tile_pool(name="pp_attn", bufs=1, space="PSUM") as pp_attn,
        ):
            # ---------- residents ----------
            q16_res = [res.tile([128, HW], F16, tag=f"q{t}", name=f"q{t}")
                       for t in range(NT)]
            k16_res = [res.tile([128, HW], F16, tag=f"k{t}", name=f"k{t}")
                       for t in range(NT)]
            vT_res = [res.tile([128, C], F16, tag=f"vT{m}", name=f"vT{m}")
                      for m in range(NQT)]
            wq_sb = [res.tile([128, C], F16, tag=f"wq{t}", name=f"wq{t}")
                     for t in range(NT)]
            wk_sb = [res.tile([128, C], F16, tag=f"wk{t}", name=f"wk{t}")
                     for t in range(NT)]
            wv_sb = [res.tile([128, C], F16, tag=f"wv{t}", name=f"wv{t}")
                     for t in range(NT)]
            wo_sb = [res.tile([128, C], F16, tag=f"wo{t}", name=f"wo{t}")
                     for t in range(NT)]
            for t in range(NT):
                sl = slice(128 * t, 128 * (t + 1))
                nc.sync.dma_start(out=wq_sb[t], in_=wq_in[sl, :])
                nc.sync.dma_start(out=wk_sb[t], in_=wk_in[sl, :])
                nc.gpsimd.dma_start(out=wv_sb[t], in_=wv_in[sl, :])
                nc.gpsimd.dma_start(out=wo_sb[t], in_=wo_in[sl, :])
            biases = res.tile([128, 16], F32, tag="biases")
            nc.sync.dma_start(out=biases, in_=bias_in[:])
            bq = biases[:, 0:4]
            bk = biases[:, 4:8]
            bv = biases[:, 8:12]
            bo = biases[:, 12:16]
            gmbt = res.tile([128, 8], F32, tag="gmbt")
            nc.sync.dma_start(out=gmbt, in_=gb_in[:])
            gam = gmbt[:, 0:4]
            bet = gmbt[:, 4:8]
            a16_sb = res.tile([128, 8], F32, tag="a16")
            nc.sync.dma_start(out=a16_sb, in_=a16_d[:])
            b8_sb = res.tile([8, 128], F32, tag="b8")
            nc.sync.dma_start(out=b8_sb, in_=b8_d[:])
            id_sb = res.tile([128, 128], F32, tag="ident")
            nc.sync.dma_start(out=id_sb, in_=ident_d[:])
            id16_sb = res.tile([128, 128], F16, tag="ident16")
            nc.gpsimd.dma_start(out=id16_sb, in_=ident16_d[:])
            eps8 = res.tile([8, 1], F32, tag="eps8")
            nc.vector.memset(eps8, EPS)
            scale_sb = res.tile([128, NT], F32, tag="scale")
            shift_sb = res.tile([128, NT], F32, tag="shift")

            # PE warmup: ~5us of dummy transposes while pass0 stats run,
            # so HAM unthrottles (1.2->2.4GHz) before passA matmuls start.
            wps = pp_log.tile([128, 128], F32, tag="ps_l", name="wps")
            for _ in range(24):
                nc.tensor.transpose(wps, id_sb, id_sb)

            # ---------- pass 0: GroupNorm statistics ----------
            with tc.tile_pool(name="p0", bufs=4) as p0, \
                 tc.tile_pool(name="p0s", bufs=1) as p0s:
                st6 = p0s.tile([128, NT, NCH, 6], F32, tag="st6")
                for n in range(NCH):
                    for t in range(NT):
                        xc = p0.tile([128, 512], F32, tag="x0")
                        nc.sync.dma_start(
                            out=xc,
                            in_=x_in[128 * t:128 * (t + 1), 512 * n:512 * (n + 1)])
                        nc.vector.bn_stats(out=st6[:, t, n, :], in_=xc)
                mv = p0s.tile([128, NT, 2], F32, tag="mv")
                for t in range(NT):
                    nc.vector.bn_aggr(out=mv[:, t, :], in_=st6[:, t, :, :])
                # stats_in: cols 0-3 mean_t, cols 4-7 E[x^2]_t
                stats_in = p0s.tile([128, 8], F32, tag="stats_in")
                for t in range(NT):
                    nc.vector.tensor_copy(stats_in[:, t:t + 1], mv[:, t, 0:1])
                    nc.vector.tensor_mul(stats_in[:, 4 + t:5 + t],
                                         mv[:, t, 0:1], mv[:, t, 0:1])
                    nc.vector.tensor_add(stats_in[:, 4 + t:5 + t],
                                         stats_in[:, 4 + t:5 + t], mv[:, t, 1:2])
                ps_g = pp_proj.tile([8, 8], F32, tag="ps_proj")
                nc.tensor.matmul(ps_g, a16_sb, stats_in, start=True, stop=True)
                g_sb = p0s.tile([8, 8], F32, tag="g_sb")
                nc.vector.tensor_copy(g_sb, ps_g)
                # group var = E[x^2]_g - mean_g^2 ; rstd = exp(-0.5*ln(var+eps))
                var_g = p0s.tile([8, 4], F32, tag="var_g")
                nc.vector.tensor_mul(var_g, g_sb[:, 0:4], g_sb[:, 0:4])
                nc.vector.tensor_tensor(out=var_g, in0=g_sb[:, 4:8], in1=var_g,
                                        op=OP.subtract)
                bc_in = p0s.tile([8, 8], F32, tag="bc_in")
                nc.vector.tensor_copy(bc_in[:, 0:4], g_sb[:, 0:4])
                nc.scalar.activation(out=bc_in[:, 4:8], in_=var_g, func=AF.Ln,
                                     bias=eps8, scale=1.0)
                nc.scalar.activation(out=bc_in[:, 4:8], in_=bc_in[:, 4:8],
                                     func=AF.Exp, bias=0.0, scale=-0.5)
                ps_bc = pp_proj.tile([128, 8], F32, tag="ps_proj")
                nc.tensor.matmul(ps_bc, b8_sb, bc_in, start=True, stop=True)
                chan = p0s.tile([128, 8], F32, tag="chan")
                nc.vector.tensor_copy(chan, ps_bc)
                # scale = gamma * rstd ; shift = beta - mean*scale
                nc.vector.tensor_mul(scale_sb, gam, chan[:, 4:8])
                tmp = p0s.tile([128, NT], F32, tag="tmp")
                nc.vector.tensor_mul(tmp, chan[:, 0:4], scale_sb)
                nc.vector.tensor_tensor(out=shift_sb, in0=bet, in1=tmp,
                                        op=OP.subtract)

            # ---------- pass A: hidden(fp16) -> q16, k16, vT (all resident) --
            with tc.tile_pool(name="pa_x", bufs=3) as pa_x, \
                 tc.tile_pool(name="pa_h", bufs=8) as pa_h:
                for n in range(NCH):
                    cols = slice(512 * n, 512 * (n + 1))
                    h16 = []
                    for t in range(NT):
                        xc = pa_x.tile([128, 512], F32, tag="xA")
                        nc.sync.dma_start(
                            out=xc, in_=x_in[128 * t:128 * (t + 1), cols])
                        h = pa_h.tile([128, 512], F16, tag="hid", bufs=5)
                        nc.vector.tensor_scalar(
                            out=h, in0=xc,
                            scalar1=scale_sb[:, t:t + 1],
                            scalar2=shift_sb[:, t:t + 1],
                            op0=OP.mult, op1=OP.add)
                        h16.append(h)
                    # vT (fp16): out[hw_t 128, c 512] = hidden_chunk_t.T @ wvT
                    # (bv folded into attn_h later: softmax weights sum to 1)
                    for t in range(NT):
                        ps = pp_proj.tile([128, 512], F32, tag="ps_proj")
                        for kc in range(NT):
                            nc.tensor.matmul(
                                ps, h16[kc][:, 128 * t:128 * (t + 1)],
                                wv_sb[kc], start=(kc == 0), stop=(kc == 3))
                        nc.vector.tensor_copy(vT_res[NT * n + t], ps)
                    # q and k projections: single-pass fp16, bias+cast on DVE
                    for (w_sb, b_col, dst) in ((wq_sb, bq, q16_res),
                                               (wk_sb, bk, k16_res)):
                        for m in range(NT):
                            ms = slice(128 * m, 128 * (m + 1))
                            ps = pp_proj.tile([128, 512], F32, tag="ps_proj")
                            for kc in range(NT):
                                nc.tensor.matmul(
                                    ps, w_sb[kc][:, ms], h16[kc],
                                    start=(kc == 0), stop=(kc == 3))
                            nc.vector.tensor_scalar_add(
                                out=dst[m][:, cols], in0=ps,
                                scalar1=b_col[:, m:m + 1])

            # ---------- attention (software-pipelined across q-tiles) -------
            with tc.tile_pool(name="at_p", bufs=10) as at_p, \
                 tc.tile_pool(name="at_pt", bufs=3) as at_pt, \
                 tc.tile_pool(name="at_s", bufs=3) as at_s, \
                 tc.tile_pool(name="at_a", bufs=2) as at_a, \
                 tc.tile_pool(name="at_h4", bufs=2) as at_h4, \
                 tc.tile_pool(name="at_o", bufs=2) as at_o:

                state = {}

                def emit_logits(qt):
                    qcols = slice(128 * qt, 128 * (qt + 1))
                    maxs = at_s.tile([128, NCH], F32, tag="maxs")
                    negms = at_s.tile([128, NCH], F32, tag="negms")
                    sums = at_s.tile([128, NCH], F32, tag="sums")
                    probs = []
                    for n in range(NCH):
                        ncols = slice(512 * n, 512 * (n + 1))
                        ps_l = pp_log.tile([128, 512], F32, tag="ps_l")
                        for kc in range(NT):
                            nc.tensor.matmul(
                                ps_l, q16_res[kc][:, qcols],
                                k16_res[kc][:, ncols],
                                start=(kc == 0), stop=(kc == 3))
                        nc.vector.reduce_max(out=maxs[:, n:n + 1], in_=ps_l,
                                             axis=AX.X)
                        nc.vector.tensor_scalar_mul(
                            out=negms[:, n:n + 1], in0=maxs[:, n:n + 1],
                            scalar1=-SCALE)
                        pr = at_p.tile([128, 512], F16, tag="probs")
                        nc.scalar.activation(
                            out=pr, in_=ps_l, func=AF.Exp,
                            bias=negms[:, n:n + 1], scale=SCALE,
                            accum_out=sums[:, n:n + 1])
                        probs.append(pr)
                    state[qt] = (maxs, sums, probs)

                emit_logits(0)
                for qt in range(NQT):
                    maxs, sums, probs = state.pop(qt)
                    # correction: c_n = exp(SCALE*(m_n - M)); needs only maxes
                    negM = at_s.tile([128, 1], F32, tag="negM")
                    nc.vector.reduce_max(out=negM, in_=maxs, axis=AX.X,
                                         negate=True)
                    negM_s = at_s.tile([128, 1], F32, tag="negM_s")
                    nc.vector.tensor_scalar_mul(out=negM_s, in0=negM,
                                                scalar1=SCALE)
                    corr = at_s.tile([128, NCH], F32, tag="corr")
                    nc.scalar.activation(out=corr, in_=maxs, func=AF.Exp,
                                         bias=negM_s, scale=SCALE)
                    for n in range(NCH):
                        nc.vector.tensor_scalar_mul(
                            out=probs[n], in0=probs[n],
                            scalar1=corr[:, n:n + 1])
                    # rowsum (needs chunk sums): rinv folded into attn scale
                    sums_c = at_s.tile([128, NCH], F32, tag="sums_c")
                    nc.vector.tensor_mul(sums_c, sums, corr)
                    rowsum = at_s.tile([128, 1], F32, tag="rowsum")
                    nc.vector.reduce_sum(out=rowsum, in_=sums_c, axis=AX.X)
                    rinv = at_s.tile([128, 1], F32, tag="rinv")
                    nc.vector.reciprocal(out=rinv, in_=rowsum)
                    # probs^T then attn@v, accumulated over all chunks
                    ps_at = pp_attn.tile([128, C], F32, tag="ps_at")
                    for n in range(NCH):
                        ps_t = pp_tr.tile([128, 512], F16, tag="ps_t")
                        for j in range(4):
                            nc.tensor.transpose(
                                ps_t[:, 128 * j:128 * (j + 1)],
                                probs[n][:, 128 * j:128 * (j + 1)], id16_sb)
                        pT = at_pt.tile([128, 512], F16, tag="pT")
                        nc.vector.tensor_copy(pT, ps_t)
                        for j in range(4):
                            nc.tensor.matmul(
                                ps_at, pT[:, 128 * j:128 * (j + 1)],
                                vT_res[NT * n + j],
                                start=(n == 0 and j == 0),
                                stop=(n == 7 and j == 3))
                    # normalize by rowsum, cast fp16, transpose to [c, q]
                    attn16 = at_a.tile([128, C], F16, tag="attn16")
                    nc.vector.tensor_scalar_mul(out=attn16, in0=ps_at,
                                                scalar1=rinv)
                    ps_t2 = pp_tr.tile([128, 512], F16, tag="ps_t")
                    for i in range(NT):
                        nc.tensor.transpose(
                            ps_t2[:, 128 * i:128 * (i + 1)],
                            attn16[:, 128 * i:128 * (i + 1)], id16_sb)
                    qq = qt % 4
                    if qq == 0:
                        attnh4 = at_h4.tile([128, NT, 512], F16, tag="attnh4")
                        state['h4'] = attnh4
                    else:
                        attnh4 = state['h4']
                    for i in range(NT):
                        nc.vector.tensor_scalar_add(
                            out=attnh4[:, i, 128 * qq:128 * (qq + 1)],
                            in0=ps_t2[:, 128 * i:128 * (i + 1)],
                            scalar1=bv[:, i:i + 1])
                    # batched output projection + bias + residual
                    if qq == 3:
                        g = qt // 4
                        gcols = slice(512 * g, 512 * (g + 1))
                        for m in range(NT):
                            ps_o = pp_proj.tile([128, 512], F32, tag="ps_proj")
                            for kc in range(NT):
                                nc.tensor.matmul(
                                    ps_o, wo_sb[kc][:, 128 * m:128 * (m + 1)],
                                    attnh4[:, kc, :], start=(kc == 0),
                                    stop=(kc == 3))
                            o_sb = at_o.tile([128, 512], F32, tag="o_sb")
                            nc.vector.tensor_scalar_add(
                                out=o_sb, in0=ps_o, scalar1=bo[:, m:m + 1])
                            xres = at_o.tile([128, 512], F32, tag="xres")
                            nc.sync.dma_start(
                                out=xres,
                                in_=x_in[128 * m:128 * (m + 1), gcols])
                            nc.vector.tensor_add(out=o_sb, in0=o_sb, in1=xres)
                            nc.sync.dma_start(
                                out=out_dram[128 * m:128 * (m + 1), gcols],
                                in_=o_sb)
                    # next q-tile's logits last: PE chews on them while
                    # vector/scalar engines prep this iteration's tail
                    if qt + 1 < NQT:
                        emit_logits(qt + 1)

    nc.compile()
    return nc


_NC_CACHE = None


def _prep_inputs(inputs):
    x = np.asarray(inputs["x"], np.float32)

    def tile4(v):
        return np.asarray(v, np.float32).reshape(4, 128).T

    biases = np.concatenate(
        [tile4(inputs[k]) for k in ("bq", "bk", "bv", "bo")], axis=1)
    gb = np.concatenate(
        [tile4(inputs["gn_gamma"]), tile4(inputs["gn_beta"])], axis=1)
    shared = {
        "wqT": np.ascontiguousarray(
            np.asarray(inputs["wq"], np.float32).T).astype(np.float16),
        "wkT": np.ascontiguousarray(
            np.asarray(inputs["wk"], np.float32).T).astype(np.float16),
        "wvT": np.ascontiguousarray(
            np.asarray(inputs["wv"], np.float32).T).astype(np.float16),
        "woT": np.ascontiguousarray(
            np.asarray(inputs["wo"], np.float32).T).astype(np.float16),
        "biases": np.ascontiguousarray(biases),
        "gammabeta": np.ascontiguousarray(gb),
    }
    return [dict(shared, x=np.ascontiguousarray(x[i].reshape(C, HW)))
            for i in range(x.shape[0])]


def kernel(**inputs):
    global _NC_CACHE
    if _NC_CACHE is None:
        _NC_CACHE = build()
    nc = _NC_CACHE
    x = np.asarray(inputs["x"], np.float32)
    b, c, h, w = x.shape
    in_maps = _prep_inputs(inputs)
    res = run_bass_kernel_spmd(nc, in_maps, list(range(b)))
    out = np.stack([res.results[i]["out"].reshape(c, h, w) for i in range(b)])
    return out.astype(np.float32)


if __name__ == "__main__":
    import time
    t0 = time.time()
    build()
    print(f"build ok in {time.time()-t0:.1f}s")
